# revision 3
# baseline (speedup 1.0000x reference)
"""Expert-parallel MoE layer for 8 Trainium2 NeuronCores.

Strategy: each of the 8 experts is assigned to one core. The host computes
the routing (which tokens go to which expert and with what combined weight),
gathers + transposes each expert's tokens into a padded [D, C] activation
matrix, and each core runs a fused  gelu(x @ W1 + b1) @ W2 + b2  kernel for
its expert, scaling each token's output column by the combine weight. The
host scatter-adds the per-expert outputs back into the full [B, S, D] output.

Matmuls run as float32r (full fp32 storage, reduced-precision PE multiply) at
1 column/cycle -- 4x faster than true fp32 matmul on TRN2.
"""

import sys

if "/opt/trn_rl_repo" not in sys.path:
    sys.path.insert(0, "/opt/trn_rl_repo")

import numpy as np

import concourse.bass as bass
import concourse.tile as tile
from concourse import bacc, mybir
from concourse.bass_utils import run_bass_kernel_spmd

B, S, D, F, E, TOPK = 4, 2048, 512, 1024, 8, 2
T = B * S
F32 = mybir.dt.float32
F32R = mybir.dt.float32r

DC = D // 128  # 4 contraction chunks for x @ W1
FC = F // 128  # 8 contraction chunks for h @ W2

# Set by test harness to capture a profile; harness-invisible otherwise.
TRACE = False
LAST_RESULTS = None

_nc_cache = {}


def _token_tiles(C):
    """Split C token columns into matmul free-dim tiles (512s + one tail)."""
    sizes = [512] * (C // 512)
    if C % 512:
        sizes.append(C % 512)
    return sizes


def _build_nc(C):
    nc = bacc.Bacc("TRN2", num_devices=E)

    xt = nc.dram_tensor("xt", [D, C], F32, kind="ExternalInput")
    w1 = nc.dram_tensor("w1", [D, F], F32, kind="ExternalInput")
    b1 = nc.dram_tensor("b1", [F, 1], F32, kind="ExternalInput")
    w2 = nc.dram_tensor("w2", [F, D], F32, kind="ExternalInput")
    b2 = nc.dram_tensor("b2", [D, 1], F32, kind="ExternalInput")
    cw = nc.dram_tensor("cw", [1, C], F32, kind="ExternalInput")
    yt = nc.dram_tensor("yt", [D, C], F32, kind="ExternalOutput")

    # DRAM views with the 128-partition chunk dim split out
    xt_r = xt.rearrange("(c p) t -> p c t", p=128)  # [128, DC, C]
    w1_r = w1.rearrange("(c p) f -> p c f", p=128)  # [128, DC, F]
    w2_r = w2.rearrange("(c p) d -> p c d", p=128)  # [128, FC, D]
    b1_r = b1.rearrange("(c p) o -> p (c o)", p=128)  # [128, FC]
    b2_r = b2.rearrange("(c p) o -> p (c o)", p=128)  # [128, DC]
    yt_r = yt.rearrange("(c p) t -> p c t", p=128)  # [128, DC, C]

    sizes = _token_tiles(C)

    with tile.TileContext(nc) as tc:
        with (
            tc.tile_pool(name="consts", bufs=1) as consts,
            tc.tile_pool(name="xtp", bufs=1) as xtp,
            tc.tile_pool(name="hp", bufs=16) as hp,
            tc.tile_pool(name="yp", bufs=4) as yp,
            tc.tile_pool(name="ybig", bufs=2) as ybigp,
            tc.tile_pool(name="ps_h", bufs=4, space="PSUM") as ps_h,
            tc.tile_pool(name="ps_y", bufs=4, space="PSUM") as ps_y,
        ):
            w1_sb = consts.tile([128, DC, F], F32R, tag="w1")
            nc.sync.dma_start(out=w1_sb[:, :, :], in_=w1_r[:, :, :].bitcast(F32R))

            # Per-token-tile slices of x^T so matmuls can start before the
            # whole activation matrix has landed.
            xt_tiles = []
            off = 0
            for i, nt in enumerate(sizes):
                xti = xtp.tile([128, DC, nt], F32R, tag=f"xt{i}")
                nc.sync.dma_start(out=xti[:, :, :], in_=xt_r[:, :, off:off + nt].bitcast(F32R))
                xt_tiles.append(xti)
                off += nt

            b1_sb = consts.tile([128, FC], F32, tag="b1")
            nc.sync.dma_start(out=b1_sb[:, :], in_=b1_r[:, :])
            b2_sb = consts.tile([128, DC], F32, tag="b2")
            nc.sync.dma_start(out=b2_sb[:, :], in_=b2_r[:, :])

            # combine weights broadcast to all 128 partitions during DMA
            cwb = consts.tile([128, C], F32, tag="cw")
            cw_ap = cw[:, :]
            cw_bcast = bass.AP(
                tensor=cw_ap.tensor,
                offset=cw_ap.offset,
                ap=[[0, 128], cw_ap.ap[-1]],
            )
            nc.gpsimd.dma_start(out=cwb[:, :], in_=cw_bcast)

            w2_sb = consts.tile([128, FC, D], F32R, tag="w2")
            nc.sync.dma_start(out=w2_sb[:, :, :], in_=w2_r[:, :, :].bitcast(F32R))

            off = 0
            for i, nt in enumerate(sizes):
                xti = xt_tiles[i]
                # h^T tile per F-chunk: [128 f, nt tokens]
                h_tiles = []
                for fc in range(FC):
                    ps = ps_h.tile([128, nt], F32, tag="psh")
                    for dc in range(DC):
                        nc.tensor.matmul(
                            ps[:, :],
                            w1_sb[:, dc, fc * 128:(fc + 1) * 128],
                            xti[:, dc, :],
                            start=(dc == 0),
                            stop=(dc == DC - 1),
                        )
                    h = hp.tile([128, nt], F32R, tag="h")
                    nc.scalar.activation(
                        h[:, :], ps[:, :],
                        mybir.ActivationFunctionType.Gelu_apprx_tanh,
                        bias=b1_sb[:, fc:fc + 1],
                    )
                    h_tiles.append(h)

                ybig = ybigp.tile([128, DC, nt], F32, tag="ybig")
                for dc in range(DC):
                    ps2 = ps_y.tile([128, nt], F32, tag="psy")
                    for fc in range(FC):
                        nc.tensor.matmul(
                            ps2[:, :],
                            w2_sb[:, fc, dc * 128:(dc + 1) * 128],
                            h_tiles[fc][:, :],
                            start=(fc == 0),
                            stop=(fc == FC - 1),
                        )
                    ytmp = yp.tile([128, nt], F32, tag="ytmp")
                    nc.scalar.activation(
                        ytmp[:, :], ps2[:, :],
                        mybir.ActivationFunctionType.Identity,
                        bias=b2_sb[:, dc:dc + 1],
                    )
                    nc.vector.tensor_mul(
                        ybig[:, dc, :], ytmp[:, :], cwb[:, off:off + nt]
                    )
                nc.sync.dma_start(out=yt_r[:, :, off:off + nt], in_=ybig[:, :, :])
                off += nt

    nc.finalize()
    return nc


def kernel(hidden, top_k_indices, top_k_weights, W1, b1, W2, b2):
    global LAST_RESULTS
    x = np.ascontiguousarray(np.asarray(hidden, dtype=np.float32).reshape(T, D))
    idx = np.asarray(top_k_indices).reshape(T, TOPK)
    w = np.asarray(top_k_weights, dtype=np.float32).reshape(T, TOPK)
    W1 = np.asarray(W1, dtype=np.float32)
    b1 = np.asarray(b1, dtype=np.float32)
    W2 = np.asarray(W2, dtype=np.float32)
    b2 = np.asarray(b2, dtype=np.float32)

    # Host routing: token lists + combined weights per expert
    tok_lists, cw_lists = [], []
    for e in range(E):
        m = idx == e
        toks = np.nonzero(m.any(axis=1))[0]
        cw_t = (w * m).sum(axis=1)[toks]
        tok_lists.append(toks)
        cw_lists.append(cw_t)

    maxn = max(len(t) for t in tok_lists)
    C = max(512, -(-maxn // 256) * 256)

    if C not in _nc_cache:
        _nc_cache[C] = _build_nc(C)
    nc = _nc_cache[C]

    in_maps = []
    for e in range(E):
        toks = tok_lists[e]
        n = len(toks)
        xt = np.zeros((D, C), np.float32)
        xt[:, :n] = x[toks].T
        cw_arr = np.zeros((1, C), np.float32)
        cw_arr[0, :n] = cw_lists[e]
        in_maps.append({
            "xt": xt,
            "w1": np.ascontiguousarray(W1[e]),
            "b1": np.ascontiguousarray(b1[e].reshape(F, 1)),
            "w2": np.ascontiguousarray(W2[e]),
            "b2": np.ascontiguousarray(b2[e].reshape(D, 1)),
            "cw": cw_arr,
        })

    kwargs = {}
    if TRACE:
        kwargs = dict(trace=True, trace_cores=list(range(E)))
    res = run_bass_kernel_spmd(nc, in_maps, core_ids=list(range(E)), **kwargs)
    LAST_RESULTS = res

    out = np.zeros((T, D), np.float32)
    for e in range(E):
        toks = tok_lists[e]
        n = len(toks)
        out[toks] += res.results[e]["yt"][:, :n].T
    return out.reshape(B, S, D)


# revision 5
# speedup vs baseline: 1.0646x; 1.0646x over previous
"""Expert-parallel MoE layer for 8 Trainium2 NeuronCores.

Strategy: each of the 8 experts is assigned to one core. The host computes
the routing (which tokens go to which expert and with what combined weight),
gathers + transposes each expert's tokens into a padded [D, C] activation
matrix, and each core runs a fused  gelu(x @ W1 + b1) @ W2 + b2  kernel for
its expert, scaling each token's output column by the combine weight. The
host scatter-adds the per-expert outputs back into the full [B, S, D] output.

Matmuls run as float32r (full fp32 storage, reduced-precision PE multiply) at
1 column/cycle -- 4x faster than true fp32 matmul on TRN2.
"""

import sys

if "/opt/trn_rl_repo" not in sys.path:
    sys.path.insert(0, "/opt/trn_rl_repo")

import numpy as np

import concourse.bass as bass
import concourse.tile as tile
from concourse import bacc, mybir
from concourse.bass_utils import run_bass_kernel_spmd

B, S, D, F, E, TOPK = 4, 2048, 512, 1024, 8, 2
T = B * S
F32 = mybir.dt.float32
F32R = mybir.dt.float32r

DC = D // 128  # 4 contraction chunks for x @ W1
FC = F // 128  # 8 contraction chunks for h @ W2

# Set by test harness to capture a profile; harness-invisible otherwise.
TRACE = False
LAST_RESULTS = None

_nc_cache = {}


def _token_tiles(C):
    """Split C token columns into matmul free-dim tiles (512s + one tail)."""
    sizes = [512] * (C // 512)
    if C % 512:
        sizes.append(C % 512)
    return sizes


def _build_nc(C):
    nc = bacc.Bacc("TRN2", num_devices=E)

    xt = nc.dram_tensor("xt", [D, C], F32, kind="ExternalInput")
    w1 = nc.dram_tensor("w1", [D, F], F32, kind="ExternalInput")
    b1 = nc.dram_tensor("b1", [F, 1], F32, kind="ExternalInput")
    w2 = nc.dram_tensor("w2", [F, D], F32, kind="ExternalInput")
    b2 = nc.dram_tensor("b2", [D, 1], F32, kind="ExternalInput")
    cw = nc.dram_tensor("cw", [1, C], F32, kind="ExternalInput")
    yt = nc.dram_tensor("yt", [D, C], F32, kind="ExternalOutput")

    # DRAM views with the 128-partition chunk dim split out
    xt_r = xt.rearrange("(c p) t -> p c t", p=128)  # [128, DC, C]
    w1_r = w1.rearrange("(c p) (g f) -> p g c f", p=128, f=128)  # [128, FC, DC, 128]
    w2_r = w2.rearrange("(c p) (g d) -> p g c d", p=128, d=128)  # [128, DC, FC, 128]
    b1_r = b1.rearrange("(c p) o -> p (c o)", p=128)  # [128, FC]
    b2_r = b2.rearrange("(c p) o -> p (c o)", p=128)  # [128, DC]
    yt_r = yt.rearrange("(c p) t -> p c t", p=128)  # [128, DC, C]

    sizes = _token_tiles(C)

    with tile.TileContext(nc) as tc:
        with (
            tc.tile_pool(name="consts", bufs=1) as consts,
            tc.tile_pool(name="xtp", bufs=1) as xtp,
            tc.tile_pool(name="hp", bufs=16) as hp,
            tc.tile_pool(name="yp", bufs=4) as yp,
            tc.tile_pool(name="ybig", bufs=8) as ybigp,
            tc.tile_pool(name="ps_h", bufs=4, space="PSUM") as ps_h,
            tc.tile_pool(name="ps_y", bufs=4, space="PSUM") as ps_y,
        ):
            # w1 grouped by output (F) chunk so the very first matmul group
            # only needs a 256 KB slice; slices stream in ahead of the PE.
            w1_sb = consts.tile([128, FC, DC, 128], F32R, tag="w1")
            nc.sync.dma_start(
                out=w1_sb[:, 0, :, :], in_=w1_r[:, 0, :, :].bitcast(F32R)
            )

            # First token tile of x^T right behind it.
            xt_tiles = [None] * len(sizes)
            offs = []
            off = 0
            for i, nt in enumerate(sizes):
                offs.append(off)
                off += nt
            xt_tiles[0] = xtp.tile([128, DC, sizes[0]], F32R, tag="xt0", name="xt_sb0")
            nc.sync.dma_start(
                out=xt_tiles[0][:, :, :],
                in_=xt_r[:, :, offs[0]:offs[0] + sizes[0]].bitcast(F32R),
            )

            for g in range(1, FC):
                nc.sync.dma_start(
                    out=w1_sb[:, g, :, :], in_=w1_r[:, g, :, :].bitcast(F32R)
                )

            b1_sb = consts.tile([128, FC], F32, tag="b1")
            nc.sync.dma_start(out=b1_sb[:, :], in_=b1_r[:, :])
            b2_sb = consts.tile([128, DC], F32, tag="b2")
            nc.sync.dma_start(out=b2_sb[:, :], in_=b2_r[:, :])

            if len(sizes) > 1:
                xt_tiles[1] = xtp.tile([128, DC, sizes[1]], F32R, tag="xt1", name="xt_sb1")
                nc.sync.dma_start(
                    out=xt_tiles[1][:, :, :],
                    in_=xt_r[:, :, offs[1]:offs[1] + sizes[1]].bitcast(F32R),
                )

            # combine weights broadcast to all 128 partitions during DMA
            cwb = consts.tile([128, C], F32, tag="cw")
            cw_ap = cw[:, :]
            cw_bcast = bass.AP(
                tensor=cw_ap.tensor,
                offset=cw_ap.offset,
                ap=[[0, 128], cw_ap.ap[-1]],
            )
            nc.gpsimd.dma_start(out=cwb[:, :], in_=cw_bcast)

            # w2 grouped by output (D) chunk, streamed per-chunk
            w2_sb = consts.tile([128, DC, FC, 128], F32R, tag="w2")
            for g in range(DC):
                nc.sync.dma_start(
                    out=w2_sb[:, g, :, :], in_=w2_r[:, g, :, :].bitcast(F32R)
                )

            for i in range(2, len(sizes)):
                xt_tiles[i] = xtp.tile([128, DC, sizes[i]], F32R, tag=f"xt{i}", name=f"xt_sb{i}")
                nc.sync.dma_start(
                    out=xt_tiles[i][:, :, :],
                    in_=xt_r[:, :, offs[i]:offs[i] + sizes[i]].bitcast(F32R),
                )

            for i, nt in enumerate(sizes):
                off = offs[i]
                xti = xt_tiles[i]
                # h^T tile per F-chunk: [128 f, nt tokens]
                h_tiles = []
                for fc in range(FC):
                    ps = ps_h.tile([128, nt], F32, tag="psh")
                    for dc in range(DC):
                        nc.tensor.matmul(
                            ps[:, :],
                            w1_sb[:, fc, dc, :],
                            xti[:, dc, :],
                            start=(dc == 0),
                            stop=(dc == DC - 1),
                        )
                    h = hp.tile([128, nt], F32R, tag="h")
                    nc.scalar.activation(
                        h[:, :], ps[:, :],
                        mybir.ActivationFunctionType.Gelu_apprx_tanh,
                        bias=b1_sb[:, fc:fc + 1],
                    )
                    h_tiles.append(h)

                for dc in range(DC):
                    ps2 = ps_y.tile([128, nt], F32, tag="psy")
                    for fc in range(FC):
                        nc.tensor.matmul(
                            ps2[:, :],
                            w2_sb[:, dc, fc, :],
                            h_tiles[fc][:, :],
                            start=(fc == 0),
                            stop=(fc == FC - 1),
                        )
                    ytmp = yp.tile([128, nt], F32, tag="ytmp")
                    nc.scalar.activation(
                        ytmp[:, :], ps2[:, :],
                        mybir.ActivationFunctionType.Identity,
                        bias=b2_sb[:, dc:dc + 1],
                    )
                    yout = ybigp.tile([128, nt], F32, tag="yout")
                    nc.vector.tensor_mul(
                        yout[:, :], ytmp[:, :], cwb[:, off:off + nt]
                    )
                    nc.sync.dma_start(
                        out=yt_r[:, dc, off:off + nt], in_=yout[:, :]
                    )

    nc.finalize()
    return nc


def kernel(hidden, top_k_indices, top_k_weights, W1, b1, W2, b2):
    global LAST_RESULTS
    x = np.ascontiguousarray(np.asarray(hidden, dtype=np.float32).reshape(T, D))
    idx = np.asarray(top_k_indices).reshape(T, TOPK)
    w = np.asarray(top_k_weights, dtype=np.float32).reshape(T, TOPK)
    W1 = np.asarray(W1, dtype=np.float32)
    b1 = np.asarray(b1, dtype=np.float32)
    W2 = np.asarray(W2, dtype=np.float32)
    b2 = np.asarray(b2, dtype=np.float32)

    # Host routing: token lists + combined weights per expert
    tok_lists, cw_lists = [], []
    for e in range(E):
        m = idx == e
        toks = np.nonzero(m.any(axis=1))[0]
        cw_t = (w * m).sum(axis=1)[toks]
        tok_lists.append(toks)
        cw_lists.append(cw_t)

    maxn = max(len(t) for t in tok_lists)
    C = max(512, -(-maxn // 256) * 256)

    if C not in _nc_cache:
        _nc_cache[C] = _build_nc(C)
    nc = _nc_cache[C]

    in_maps = []
    for e in range(E):
        toks = tok_lists[e]
        n = len(toks)
        xt = np.zeros((D, C), np.float32)
        xt[:, :n] = x[toks].T
        cw_arr = np.zeros((1, C), np.float32)
        cw_arr[0, :n] = cw_lists[e]
        in_maps.append({
            "xt": xt,
            "w1": np.ascontiguousarray(W1[e]),
            "b1": np.ascontiguousarray(b1[e].reshape(F, 1)),
            "w2": np.ascontiguousarray(W2[e]),
            "b2": np.ascontiguousarray(b2[e].reshape(D, 1)),
            "cw": cw_arr,
        })

    kwargs = {}
    if TRACE:
        kwargs = dict(trace=True, trace_cores=list(range(E)))
    res = run_bass_kernel_spmd(nc, in_maps, core_ids=list(range(E)), **kwargs)
    LAST_RESULTS = res

    out = np.zeros((T, D), np.float32)
    for e in range(E):
        toks = tok_lists[e]
        n = len(toks)
        out[toks] += res.results[e]["yt"][:, :n].T
    return out.reshape(B, S, D)
